# revision 8
# baseline (speedup 1.0000x reference)
"""Trainium2 Bass kernel for nn_Encoder (GAT layer + BatchNorm over 16
independent 360-node graphs).

Strategy (data-parallel over graphs, 2 graphs per core on 8 cores):
  - The GAT linear weight W is identity-per-head, so h[n,hd,:] == x[n,:] and
    the whole per-graph GAT collapses to out_g = A_g @ x_g with a dense
    [360,360] attention matrix A_g built on-device.
  - Per-edge attention logits a_s[src] are delivered to edge slots via a
    "pet-slot" scheme: each source s in a graph gets a color pet(s) such that
    sources sharing a destination have distinct colors (greedy conflict
    coloring on the host, index data only).  Then gathered[n, pet(s)*4+h] =
    sum_s Struct[s,n] * Rpet[s, pet(s)*4+h] is ONE matmul; Rpet is built with
    gpsimd local_scatter from the on-device a_s values.
  - Edge-space softmax (dst-major, [120 x 6*KPET*4] batched), multiplicity
    handled by adding ln(cnt) to the logits before exp.
  - Weights scattered into A rows (local_scatter), A transposed by DMA
    transpose, aggregation matmul with bias folded in as an extra ones
    column / bias row of x_aug.
  - BatchNorm batch stats are global across all 5760 nodes: kernel 1 emits
    per-core (sum, sumsq); the 8 partial sums are all-reduced on the host
    (as sanctioned by the sharding strategy); kernel 2 applies the affine +
    the in-place diagonal zeroing.
"""

import math

import numpy as np

import concourse.bacc as bacc
import concourse.bass_utils as bass_utils
import concourse.mybir as mybir
import concourse.tile as tile

F32 = mybir.dt.float32
F16 = mybir.dt.float16
I16 = mybir.dt.int16

N_CORES = 8
NPG = 360          # nodes per graph
DIM = 360
HEADS = 4
GPC = 2            # graphs per core
NLOC = GPC * NPG   # 720 local nodes per core
NSUB = 6           # 120-row n-subtiles per core
PSUB = 120
XAUG_R = NPG + 2   # x_aug rows per graph: 360 + bias row + zero row = 362
APAD = 384         # A-matrix row length padded for 128-aligned DMA transpose
BN_EPS = 1e-5
NEG_SLOPE = 0.2
LNC_NEG = -60000.0  # ~-inf in f16: kills empty pet slots after exp

# aggregation contraction sub-tiles over the 362 A^T rows
KB_OFF = (0, 128, 256)
KB_W = (128, 128, 106)


# ----------------------------------------------------------------------------
# host-side edge preprocessing (pure index manipulation)
# ----------------------------------------------------------------------------

def _graph_edges(src, dst, g):
    """Unique (n, s, cnt) edge list of graph g (incl. self loops), local ids."""
    lo = g * NPG
    sel = (dst >= lo) & (dst < lo + NPG)
    sd = src[sel] - lo
    dd = dst[sel] - lo
    assert sd.min() >= 0 and sd.max() < NPG, "edges must be graph-local"
    sd = np.concatenate([sd, np.arange(NPG, dtype=sd.dtype)])
    dd = np.concatenate([dd, np.arange(NPG, dtype=dd.dtype)])
    key = dd.astype(np.int64) * NPG + sd
    uk, cnt = np.unique(key, return_counts=True)
    n_arr = (uk // NPG).astype(np.int32)
    s_arr = (uk % NPG).astype(np.int32)
    return n_arr, s_arr, cnt.astype(np.int32)


def _color_graph(n_arr, s_arr):
    """Greedy conflict coloring: sources sharing a destination get distinct
    colors. Returns pet[s] for s in [0, NPG)."""
    # per-destination source lists as a padded matrix
    order = np.argsort(n_arr, kind="stable")
    n_sorted, s_sorted = n_arr[order], s_arr[order]
    counts = np.bincount(n_sorted, minlength=NPG)
    maxd = counts.max()
    dmat = np.full((NPG, maxd), -1, np.int32)
    pos = np.zeros(NPG, np.int64)
    starts = np.zeros(NPG + 1, np.int64)
    np.cumsum(counts, out=starts[1:])
    for n in range(NPG):
        dmat[n, : counts[n]] = s_sorted[starts[n] : starts[n + 1]]
    # per-source destination lists
    order2 = np.argsort(s_arr, kind="stable")
    s2, n2 = s_arr[order2], n_arr[order2]
    scount = np.bincount(s2, minlength=NPG)
    sstart = np.zeros(NPG + 1, np.int64)
    np.cumsum(scount, out=sstart[1:])
    color = np.full(NPG, -1, np.int32)
    for s in np.argsort(-scount, kind="stable"):
        dsts = n2[sstart[s] : sstart[s + 1]]
        co = dmat[dsts].ravel()
        used = color[co[co >= 0]]
        used = used[used >= 0]
        if used.size:
            mask = np.zeros(used.max() + 2, bool)
            mask[used] = True
            c = int(np.argmin(mask))
        else:
            c = 0
        color[s] = c
    return color


def _prep_all(x, src, dst, att_src, att_dst, bias, gamma, beta):
    """Host preprocessing: returns (KPET, list of per-core in_maps for K1,
    shared small tensors)."""
    B = x.shape[0] // NPG
    graphs = []
    kpet = 0
    for g in range(B):
        n_arr, s_arr, cnt = _graph_edges(src, dst, g)
        pet = _color_graph(n_arr, s_arr)
        graphs.append((n_arr, s_arr, cnt, pet))
        kpet = max(kpet, int(pet.max()) + 1)
    kpet = (kpet + 1) & ~1  # even
    K4 = kpet * 4

    att_cat = np.concatenate([att_src, att_dst], axis=0)  # [8, 360]
    attT = np.ascontiguousarray(att_cat.T).astype(np.float16)  # [360, 8]

    in_maps = []
    for c in range(N_CORES):
        xl = x[c * NLOC : (c + 1) * NLOC]  # [720, 360]
        xT = np.ascontiguousarray(xl.T).astype(np.float16)
        xaug = np.zeros((GPC * XAUG_R, DIM), np.float16)
        sst = np.zeros((NLOC, NPG), np.float16)
        lnc = np.full((128, NSUB * kpet), LNC_NEG, np.float16)
        widx = np.full((128, NSUB * (kpet + 2)), -1, np.int16)
        ridx = np.full((128, NSUB * 4), -1, np.int16)
        for gl in range(GPC):
            g = c * GPC + gl
            n_arr, s_arr, cnt, pet = graphs[g]
            xaug[gl * XAUG_R : gl * XAUG_R + NPG] = xl[gl * NPG : (gl + 1) * NPG].astype(
                np.float16
            )
            xaug[gl * XAUG_R + NPG] = bias.astype(np.float16)
            sst[gl * NPG + s_arr, n_arr] = 1.0
            p_e = pet[s_arr]
            # n-subtile index (0..5) and row within it
            t_e = gl * 3 + n_arr // PSUB
            p_row = n_arr % PSUB
            lnc[p_row, t_e * kpet + p_e] = np.log(cnt).astype(np.float16)
            widx[p_row, t_e * (kpet + 2) + p_e] = s_arr
            # source-side pet slots: s-subtile index and row
            ts_e = gl * 3 + np.arange(NPG) // PSUB
            ps_row = np.arange(NPG) % PSUB
            for h in range(HEADS):
                ridx[ps_row, ts_e * 4 + h] = pet * 4 + h
        widx[:PSUB, np.arange(NSUB) * (kpet + 2) + kpet] = NPG  # bias column
        in_maps.append(
            {
                "xT": xT,
                "xaug": xaug,
                "sst": sst,
                "attT": attT,
                "lnc": lnc,
                "widx": widx,
                "ridx": ridx,
            }
        )
    return kpet, in_maps


# ----------------------------------------------------------------------------
# kernel 1: GAT + tanh + local BN partial sums
# ----------------------------------------------------------------------------

def _build_k1(kpet):
    K4 = kpet * 4
    nc = bacc.Bacc("TRN2", target_bir_lowering=False, debug=False,
                   num_devices=N_CORES)
    d_xT = nc.dram_tensor("xT", [DIM, NLOC], F16, kind="ExternalInput")
    d_xaug = nc.dram_tensor("xaug", [GPC * XAUG_R, DIM], F16, kind="ExternalInput")
    d_sst = nc.dram_tensor("sst", [NLOC, NPG], F16, kind="ExternalInput")
    d_attT = nc.dram_tensor("attT", [DIM, 8], F16, kind="ExternalInput")
    d_lnc = nc.dram_tensor("lnc", [128, NSUB * kpet], F16, kind="ExternalInput")
    d_widx = nc.dram_tensor("widx", [128, NSUB * (kpet + 2)], I16,
                            kind="ExternalInput")
    d_ridx = nc.dram_tensor("ridx", [128, NSUB * 4], I16, kind="ExternalInput")
    d_outpre = nc.dram_tensor("outpre", [NLOC, DIM], F16, kind="ExternalOutput")
    d_stats = nc.dram_tensor("stats", [1, 2 * DIM], F32, kind="ExternalOutput")

    with tile.TileContext(nc) as tc:
        with (
            tc.tile_pool(name="cst", bufs=1) as cst,
            tc.tile_pool(name="gp", bufs=2, space="PSUM") as gp,
            tc.tile_pool(name="yp", bufs=2, space="PSUM") as yp,
            tc.tile_pool(name="sp", bufs=1, space="PSUM") as sp,
        ):
            # ---- input loads ----
            xT_t = []
            for i in range(3):
                t = cst.tile([128, NLOC], F16, tag=f"xT{i}")
                nc.sync.dma_start(t[0:PSUB, :], d_xT[i * PSUB : (i + 1) * PSUB, :])
                xT_t.append(t)
            xa_t = []
            for gl in range(GPC):
                for kb in range(3):
                    w = KB_W[kb]
                    t = cst.tile([128, DIM], F16, tag=f"xa{gl}{kb}")
                    nc.sync.dma_start(
                        t[0:w, :],
                        d_xaug[gl * XAUG_R + KB_OFF[kb] : gl * XAUG_R + KB_OFF[kb] + w, :],
                    )
                    xa_t.append(t)
            sst_t = []
            for j in range(NSUB):
                t = cst.tile([128, NPG], F16, tag=f"sst{j}")
                nc.sync.dma_start(t[0:PSUB, :], d_sst[j * PSUB : (j + 1) * PSUB, :])
                sst_t.append(t)
            attT_t = []
            for i in range(3):
                t = cst.tile([128, 8], F16, tag=f"attT{i}")
                nc.sync.dma_start(t[0:PSUB, :], d_attT[i * PSUB : (i + 1) * PSUB, :])
                attT_t.append(t)
            lnc_t = cst.tile([128, NSUB * kpet], F16, tag="lnc")
            nc.sync.dma_start(lnc_t[:], d_lnc[:])
            widx_t = cst.tile([128, NSUB * (kpet + 2)], I16, tag="widx")
            nc.sync.dma_start(widx_t[:], d_widx[:])
            ridx_t = cst.tile([128, NSUB * 4], I16, tag="ridx")
            nc.sync.dma_start(ridx_t[:], d_ridx[:])

            # ---- a = [a_src | a_dst] per node:  a[n, j] = x[n] . att_cat[j]
            anat = sp.tile([128, NSUB * 8], F32, tag="anat")
            for t in range(NSUB):
                for i in range(3):
                    nc.tensor.matmul(
                        anat[0:PSUB, t * 8 : (t + 1) * 8],
                        xT_t[i][0:PSUB, t * PSUB : (t + 1) * PSUB],
                        attT_t[i][0:PSUB, :],
                        start=(i == 0),
                        stop=(i == 2),
                    )
            asb = cst.tile([128, NSUB * 8], F32, tag="asb")
            nc.vector.tensor_copy(asb[0:PSUB, :], anat[0:PSUB, :])
            af16 = cst.tile([128, NSUB * 4], F16, tag="af16")
            nc.vector.memset(af16[:], 0.0)
            nc.vector.tensor_copy(
                af16[0:PSUB, :].rearrange("p (t h) -> p t h", h=4),
                asb[0:PSUB, :].rearrange("p (t m) -> p t m", m=8)[:, :, 0:4],
            )

            # ---- Rpet: scatter a_src values to pet slots (per s-subtile) ----
            rp_t = []
            for t in range(NSUB):
                rp = cst.tile([128, K4], F16, tag=f"rp{t}")
                nc.gpsimd.local_scatter(
                    rp[:],
                    af16[:, t * 4 : (t + 1) * 4],
                    ridx_t[:, t * 4 : (t + 1) * 4],
                    channels=128,
                    num_elems=K4,
                    num_idxs=4,
                )
                rp_t.append(rp)

            # ---- delivery matmul + edge-space softmax ----
            alpha = cst.tile([128, NSUB * K4], F32, tag="alpha")
            for t in range(NSUB):
                gl = t // 3
                g_ps = gp.tile([128, K4], F32, tag="g")
                for ks in range(3):
                    j = gl * 3 + ks
                    nc.tensor.matmul(
                        g_ps[0:PSUB, :],
                        sst_t[j][0:PSUB, (t % 3) * PSUB : (t % 3 + 1) * PSUB],
                        rp_t[j][0:PSUB, :],
                        start=(ks == 0),
                        stop=(ks == 2),
                    )
                # alpha = G + a_dst (broadcast over pet slots)
                nc.vector.tensor_tensor(
                    alpha[0:PSUB, t * K4 : (t + 1) * K4].rearrange(
                        "p (k h) -> p k h", h=4
                    ),
                    g_ps[0:PSUB, :].rearrange("p (k h) -> p k h", h=4),
                    asb[0:PSUB, t * 8 + 4 : t * 8 + 8]
                    .unsqueeze(1)
                    .broadcast_to([PSUB, kpet, 4]),
                    mybir.AluOpType.add,
                )
            alpha2 = cst.tile([128, NSUB * K4], F32, tag="alpha2")
            nc.vector.scalar_tensor_tensor(
                alpha2[0:PSUB, :],
                alpha[0:PSUB, :],
                NEG_SLOPE,
                alpha[0:PSUB, :],
                mybir.AluOpType.mult,
                mybir.AluOpType.max,
            )
            alpha3 = cst.tile([128, NSUB * K4], F32, tag="alpha3")
            for t in range(NSUB):
                nc.vector.tensor_tensor(
                    alpha3[0:PSUB, t * K4 : (t + 1) * K4].rearrange(
                        "p (k h) -> p k h", h=4
                    ),
                    alpha2[0:PSUB, t * K4 : (t + 1) * K4].rearrange(
                        "p (k h) -> p k h", h=4
                    ),
                    lnc_t[0:PSUB, t * kpet : (t + 1) * kpet]
                    .unsqueeze(2)
                    .broadcast_to([PSUB, kpet, 4]),
                    mybir.AluOpType.add,
                )
            uexp = cst.tile([128, NSUB * K4], F32, tag="uexp")
            nc.scalar.activation(
                uexp[0:PSUB, :], alpha3[0:PSUB, :],
                mybir.ActivationFunctionType.Exp,
            )
            den = cst.tile([128, NSUB * 4], F32, tag="den")
            for t in range(NSUB):
                nc.vector.tensor_reduce(
                    den[0:PSUB, t * 4 : (t + 1) * 4],
                    uexp[0:PSUB, t * K4 : (t + 1) * K4]
                    .rearrange("p (k h) -> p k h", h=4)
                    .transpose([0, 2, 1]),
                    axis=mybir.AxisListType.X,
                    op=mybir.AluOpType.add,
                )
            rec = cst.tile([128, NSUB * 4], F32, tag="rec")
            nc.vector.reciprocal(rec[0:PSUB, :], den[0:PSUB, :])
            wp = cst.tile([128, NSUB * K4], F32, tag="wp")
            for t in range(NSUB):
                nc.vector.scalar_tensor_tensor(
                    wp[0:PSUB, t * K4 : (t + 1) * K4].rearrange(
                        "p (k h) -> p k h", h=4
                    ),
                    uexp[0:PSUB, t * K4 : (t + 1) * K4].rearrange(
                        "p (k h) -> p k h", h=4
                    ),
                    1.0 / HEADS,
                    rec[0:PSUB, t * 4 : (t + 1) * 4]
                    .unsqueeze(1)
                    .broadcast_to([PSUB, kpet, 4]),
                    mybir.AluOpType.mult,
                    mybir.AluOpType.mult,
                )
            ws = cst.tile([128, NSUB * kpet], F32, tag="ws")
            for t in range(NSUB):
                nc.vector.tensor_reduce(
                    ws[0:PSUB, t * kpet : (t + 1) * kpet],
                    wp[0:PSUB, t * K4 : (t + 1) * K4].rearrange(
                        "p (k h) -> p k h", h=4
                    ),
                    axis=mybir.AxisListType.X,
                    op=mybir.AluOpType.add,
                )
            wdata = cst.tile([128, NSUB * (kpet + 2)], F16, tag="wdata")
            nc.vector.memset(wdata[:], 0.0)
            nc.vector.tensor_copy(
                wdata[0:PSUB, :].rearrange("p (t m) -> p t m", m=kpet + 2)[
                    :, :, 0:kpet
                ],
                ws[0:PSUB, :].rearrange("p (t k) -> p t k", t=NSUB),
            )
            nc.vector.memset(
                wdata[:, :].rearrange("p (t m) -> p t m", m=kpet + 2)[
                    :, :, kpet : kpet + 1
                ],
                1.0,
            )

            # ---- A rows scatter, transpose, aggregation ----
            at_t = []
            for g in range(GPC):
                for kb in range(3):
                    t = cst.tile([128, 384], F16, tag=f"at{g}{kb}")
                    at_t.append(t)
            for t in range(NSUB):
                a_t = cst.tile([128, APAD], F16, tag=f"a{t}")
                nc.gpsimd.local_scatter(
                    a_t[:],
                    wdata[:, t * (kpet + 2) : (t + 1) * (kpet + 2)],
                    widx_t[:, t * (kpet + 2) : (t + 1) * (kpet + 2)],
                    channels=128,
                    num_elems=APAD,
                    num_idxs=kpet + 2,
                )
                gl = t // 3
                for kb in range(3):
                    nc.sync.dma_start_transpose(
                        at_t[gl * 3 + kb][0:128, (t % 3) * 128 : (t % 3 + 1) * 128],
                        a_t[0:128, kb * 128 : (kb + 1) * 128],
                    )

            ones16 = cst.tile([128, 1], F16, tag="ones16")
            nc.vector.memset(ones16[:], 1.0)
            s_ps = sp.tile([1, DIM], F32, tag="sum")
            s2_ps = sp.tile([1, DIM], F32, tag="sumsq")
            for t in range(NSUB):
                gl = t // 3
                y_ps = yp.tile([128, DIM], F32, tag="y")
                for kb in range(3):
                    w = KB_W[kb]
                    nc.tensor.matmul(
                        y_ps[0:128, :],
                        at_t[gl * 3 + kb][0:w, (t % 3) * 128 : (t % 3) * 128 + 128],
                        xa_t[gl * 3 + kb][0:w, :],
                        start=(kb == 0),
                        stop=(kb == 2),
                    )
                op_t = cst.tile([128, DIM], F16, tag=f"op{t}")
                nc.scalar.activation(
                    op_t[0:PSUB, :], y_ps[0:PSUB, :],
                    mybir.ActivationFunctionType.Tanh,
                )
                nc.sync.dma_start(
                    d_outpre[t * PSUB : (t + 1) * PSUB, :], op_t[0:PSUB, :]
                )
                sq_t = cst.tile([128, DIM], F16, tag=f"sq{t}")
                nc.vector.tensor_tensor(
                    sq_t[0:PSUB, :], op_t[0:PSUB, :], op_t[0:PSUB, :],
                    mybir.AluOpType.mult,
                )
                nc.tensor.matmul(
                    s_ps[0:1, :], ones16[0:PSUB, :], op_t[0:PSUB, :],
                    start=(t == 0), stop=(t == NSUB - 1),
                )
                nc.tensor.matmul(
                    s2_ps[0:1, :], ones16[0:PSUB, :], sq_t[0:PSUB, :],
                    start=(t == 0), stop=(t == NSUB - 1),
                )
            stats_sb = cst.tile([1, 2 * DIM], F32, tag="stats")
            nc.vector.tensor_copy(stats_sb[0:1, 0:DIM], s_ps[0:1, :])
            nc.vector.tensor_copy(stats_sb[0:1, DIM : 2 * DIM], s2_ps[0:1, :])
            nc.sync.dma_start(d_stats[:], stats_sb[:])

    nc.compile()
    return nc


# ----------------------------------------------------------------------------
# kernel 2: BN affine (global stats) + diagonal mask
# ----------------------------------------------------------------------------

def _build_k2():
    nc = bacc.Bacc("TRN2", target_bir_lowering=False, debug=False,
                   num_devices=N_CORES)
    d_outpre = nc.dram_tensor("outpre", [NLOC, DIM], F16, kind="ExternalInput")
    d_gstats = nc.dram_tensor("gstats", [1, 2 * DIM], F32, kind="ExternalInput")
    d_gb = nc.dram_tensor("gb", [1, 2 * DIM], F32, kind="ExternalInput")
    d_out = nc.dram_tensor("out", [NLOC, DIM], F32, kind="ExternalOutput")
    NTOT = float(N_CORES * NLOC)

    with tile.TileContext(nc) as tc:
        with (
            tc.tile_pool(name="cst", bufs=1) as cst,
            tc.tile_pool(name="ps", bufs=1, space="PSUM") as ps,
        ):
            op_t = []
            for t in range(NSUB):
                tt = cst.tile([128, DIM], F16, tag=f"op{t}")
                nc.sync.dma_start(
                    tt[0:PSUB, :], d_outpre[t * PSUB : (t + 1) * PSUB, :]
                )
                op_t.append(tt)
            gs = cst.tile([1, 2 * DIM], F32, tag="gs")
            nc.sync.dma_start(gs[:], d_gstats[:])
            gb = cst.tile([1, 2 * DIM], F32, tag="gb")
            nc.sync.dma_start(gb[:], d_gb[:])

            mu = cst.tile([1, DIM], F32, tag="mu")
            nc.vector.tensor_scalar_mul(mu[0:1, :], gs[0:1, 0:DIM], 1.0 / NTOT)
            ex2 = cst.tile([1, DIM], F32, tag="ex2")
            nc.vector.tensor_scalar_mul(
                ex2[0:1, :], gs[0:1, DIM : 2 * DIM], 1.0 / NTOT
            )
            var = cst.tile([1, DIM], F32, tag="var")
            # var = ex2 - mu^2  ==  (mu * -mu) + ex2
            nc.vector.tensor_tensor(var[0:1, :], mu[0:1, :], mu[0:1, :],
                                    mybir.AluOpType.mult)
            nc.vector.tensor_tensor(var[0:1, :], ex2[0:1, :], var[0:1, :],
                                    mybir.AluOpType.subtract)
            nc.vector.tensor_scalar_add(var[0:1, :], var[0:1, :], BN_EPS)
            sd = cst.tile([1, DIM], F32, tag="sd")
            nc.scalar.activation(sd[0:1, :], var[0:1, :],
                                 mybir.ActivationFunctionType.Sqrt)
            st_row = cst.tile([1, 2 * DIM], F32, tag="st_row")
            rstd = cst.tile([1, DIM], F32, tag="rstd")
            nc.vector.reciprocal(rstd[0:1, :], sd[0:1, :])
            # s = gamma * rstd ; t = beta - mu * s
            nc.vector.tensor_tensor(st_row[0:1, 0:DIM], rstd[0:1, :],
                                    gb[0:1, 0:DIM], mybir.AluOpType.mult)
            tmp = cst.tile([1, DIM], F32, tag="tmp")
            nc.vector.tensor_tensor(tmp[0:1, :], mu[0:1, :],
                                    st_row[0:1, 0:DIM], mybir.AluOpType.mult)
            nc.vector.tensor_tensor(st_row[0:1, DIM : 2 * DIM],
                                    gb[0:1, DIM : 2 * DIM], tmp[0:1, :],
                                    mybir.AluOpType.subtract)

            ones_row = cst.tile([1, 128], F32, tag="ones_row")
            nc.vector.memset(ones_row[:], 1.0)
            s_ps = ps.tile([128, DIM], F32, tag="sbb")
            t_ps = ps.tile([128, DIM], F32, tag="tbb")
            nc.tensor.matmul(s_ps[:], ones_row[0:1, :], st_row[0:1, 0:DIM])
            nc.tensor.matmul(t_ps[:], ones_row[0:1, :],
                             st_row[0:1, DIM : 2 * DIM])
            st_sb = cst.tile([128, 2 * DIM], F32, tag="st_sb")
            nc.vector.tensor_copy(st_sb[:, 0:DIM], s_ps[:])
            nc.vector.tensor_copy(st_sb[:, DIM : 2 * DIM], t_ps[:])

            for t in range(NSUB):
                tmp_t = cst.tile([128, DIM], F32, tag="tmpt")
                nc.vector.tensor_tensor(
                    tmp_t[0:PSUB, :], op_t[t][0:PSUB, :],
                    st_sb[0:PSUB, 0:DIM], mybir.AluOpType.mult,
                )
                fin_t = cst.tile([128, DIM], F32, tag="fint")
                nc.vector.tensor_tensor(
                    fin_t[0:PSUB, :], tmp_t[0:PSUB, :],
                    st_sb[0:PSUB, DIM : 2 * DIM], mybir.AluOpType.add,
                )
                msk_t = cst.tile([128, DIM], F32, tag="mskt")
                nc.gpsimd.affine_select(
                    msk_t[0:PSUB, :],
                    fin_t[0:PSUB, :],
                    pattern=[[1, DIM]],
                    compare_op=mybir.AluOpType.not_equal,
                    fill=0.0,
                    base=-(t % 3) * PSUB,
                    channel_multiplier=-1,
                )
                nc.sync.dma_start(
                    d_out[t * PSUB : (t + 1) * PSUB, :], msk_t[0:PSUB, :]
                )

    nc.compile()
    return nc


# ----------------------------------------------------------------------------
# entry point
# ----------------------------------------------------------------------------

_NC_CACHE = {}


def _get_kernels(kpet):
    if kpet not in _NC_CACHE:
        _NC_CACHE[kpet] = (_build_k1(kpet), _build_k2())
    return _NC_CACHE[kpet]


def kernel(x, edge_index, W, att_src, att_dst, bias, gamma, beta):
    x = np.asarray(x, np.float32)
    ei = np.asarray(edge_index)
    src = ei[0].astype(np.int64)
    dst = ei[1].astype(np.int64)
    W = np.asarray(W, np.float32)
    att_src = np.asarray(att_src, np.float32)
    att_dst = np.asarray(att_dst, np.float32)
    bias = np.asarray(bias, np.float32)
    gamma = np.asarray(gamma, np.float32)
    beta = np.asarray(beta, np.float32)

    # the kernel exploits W == [I I I I] (per-head identity); verify.
    W_expect = np.concatenate([np.eye(DIM, dtype=np.float32)] * HEADS, axis=1)
    assert np.array_equal(W, W_expect), "kernel assumes identity-per-head W"

    kpet, in_maps1 = _prep_all(x, src, dst, att_src, att_dst, bias, gamma, beta)
    nc1, nc2 = _get_kernels(kpet)

    res1 = bass_utils.run_bass_kernel_spmd(
        nc1, in_maps1, core_ids=list(range(N_CORES))
    )
    # host all-reduce of BN partial sums (index-free data movement only)
    gstats = np.zeros((1, 2 * DIM), np.float32)
    for r in res1.results:
        gstats += r["stats"]
    gb = np.concatenate([gamma, beta]).reshape(1, 2 * DIM).astype(np.float32)
    in_maps2 = [
        {"outpre": r["outpre"], "gstats": gstats, "gb": gb}
        for r in res1.results
    ]
    res2 = bass_utils.run_bass_kernel_spmd(
        nc2, in_maps2, core_ids=list(range(N_CORES))
    )
    out = np.concatenate([r["out"] for r in res2.results], axis=0)
    recon_x = out.reshape(-1, NPG * DIM)
    return (recon_x, out, out)
